# revision 1
# baseline (speedup 1.0000x reference)
"""Converse2D-Up (FFT deconvolution upsampler) as a Bass/Tile kernel for TRN2.

Math (validated against the jax reference to rel-l2 ~1.1e-4 == the
reference's own fp32 noise floor):

The whole pipeline before the final gelu is linear in x and channel-wise.
With xp = wrap-pad(x) (132x132), Y = FFT132(xp) = G @ x @ G^T where
G = F132 @ P (132x128, P = periodic pad selection).  The reference's
264-point spectral transfer function H (built from weight/bias only) is
Hermitian, so out = crop(real(IFFT264(H . tile(Y)))) decomposes into 4
polyphase outputs out_dd = real(IFFT132(Kdd_hat . Y)) with per-channel
precomputed spectra Kdd_hat; the crop leaves exactly 128 rows/cols per
phase.  Hermitian symmetry further means only columns v=0..66 of
Kdd_hat.Y are needed:
    T1[x,v] = sum_u Ai[x,u] (Kdd_hat.Y)[u,v]      (Ai = cropped iF132 rows)
    out[x,y] = sum_{v=0..66} w_v Re(T1[x,v] Ai[y,v]),  w = [1,2,...,2,1]
Everything maps onto fp32 PE matmuls with K<=132 contractions (split
128+4), a small pointwise complex multiply (DVE for the 128-row chunk,
GPSIMD for the 4-row chunk), and gelu+phase-interleave fused into the
ScalarE PSUM->SBUF eviction.

Sharding: 8 channels per core x 4 batch images (all per-(B,C)
independent); weight/bias-derived spectra are host-precomputed constants.
"""

import os

import numpy as np

import concourse.bass as bass
import concourse.mybir as mybir
import concourse.tile as tile
from concourse import bacc
from concourse.bass import ts
from concourse.bass_utils import run_bass_kernel_spmd

F32 = mybir.dt.float32
AF = mybir.ActivationFunctionType

SCALE = 2
PAD = 2
EPS = 1e-5
N0 = 128           # input spatial size
NP = N0 + 2 * PAD  # 132 padded
NU = NP * SCALE    # 264 upsampled
NV = NP // 2 + 1   # 67 unique spectral columns
B = 4
C = 64
NCORES = 8
CPC = C // NCORES  # 8 channels per core
NIMG = B * CPC     # 32 images per core

LAST_EXEC_NS = None  # set by kernel() when tracing is enabled


# --------------------------------------------------------------------------
# host-side constant precompute (weight/bias -> per-channel spectra)
# --------------------------------------------------------------------------

def _host_constants(weight, bias):
    w64 = np.asarray(weight, dtype=np.float64)
    b64 = np.asarray(bias, dtype=np.float64)

    # FB = p2o(weight): 264-point OTF of the rolled 3x3 PSF, per channel
    k_h, k_w = w64.shape[-2:]
    otf = np.zeros((C, NU, NU), dtype=np.complex128)
    otf[:, :k_h, :k_w] = w64[0]
    otf = np.roll(otf, (-(k_h // 2), -(k_w // 2)), axis=(-2, -1))
    FB = np.fft.fftn(otf, axes=(-2, -1))                      # (C,264,264)

    biaseps = 1.0 / (1.0 + np.exp(-(b64.reshape(C) - 9.0))) + EPS  # (C,)
    be = biaseps[:, None, None]

    u = np.arange(NU)
    Dr = 1 + np.exp(-2j * np.pi * u / NU)
    D = Dr[:, None] * Dr[None, :]                             # (264,264)

    Gh = np.conj(FB) + be * D[None]
    FBG = FB * Gh

    def quadmean(A):
        return 0.25 * (A[:, :NP, :NP] + A[:, NP:, :NP]
                       + A[:, :NP, NP:] + A[:, NP:, NP:])

    M1 = quadmean(FBG)
    invW = quadmean(np.abs(FB) ** 2)
    M2 = M1 / (invW + be)
    H = (Gh - np.conj(FB) * np.tile(M2, (1, SCALE, SCALE))) / be   # (C,264,264)

    hr = np.fft.ifft2(H, axes=(-2, -1)).real                  # H Hermitian
    # polyphase spectra: Kdd_hat[c,dx,dy] = FFT132(hr[c, dx::2, dy::2])
    kdd = np.empty((C, 2, 2, NP, NV), dtype=np.complex128)
    for dx in range(2):
        for dy in range(2):
            kh = np.fft.fft2(hr[:, dx::2, dy::2], axes=(-2, -1))
            kdd[:, dx, dy] = kh[:, :, :NV]

    # pack per channel: [u, plane(3), phase(4), v] planes = [Kr, Ki, Kr]
    kr = np.empty((C, NP, 4, NV), dtype=np.float32)
    ki = np.empty((C, NP, 4, NV), dtype=np.float32)
    for dx in range(2):
        for dy in range(2):
            p = dx * 2 + dy
            kr[:, :, p, :] = kdd[:, dx, dy].real.astype(np.float32)
            ki[:, :, p, :] = kdd[:, dx, dy].imag.astype(np.float32)
    kdd_packed = np.concatenate(
        [kr.reshape(C, NP, 4 * NV), ki.reshape(C, NP, 4 * NV),
         kr.reshape(C, NP, 4 * NV)], axis=2,
    )                                                          # (C,132,804)

    # forward matrix G = F132 @ P  (132x128 complex)
    P = np.zeros((NP, N0))
    for m in range(NP):
        P[m, (m - PAD) % N0] = 1.0
    F132 = np.exp(-2j * np.pi * np.outer(np.arange(NP), np.arange(NP)) / NP)
    G = F132 @ P

    gt = np.concatenate([G.real.T, G.imag.T], axis=1).astype(np.float32)   # (128,264)
    neg67 = (-G.imag.T[:, :NV]).astype(np.float32)                          # (128,67)

    # inverse matrix, rows i in [2,130) of iF132/132
    Ai = np.exp(2j * np.pi * np.outer(np.arange(2, 130), np.arange(NP)) / NP) / NP
    Cm, Sm = Ai.real, Ai.imag                                  # (128,132)
    CT, ST = Cm.T, Sm.T                                        # (132,128)
    cst = np.concatenate([CT, ST, -ST], axis=1).astype(np.float32)  # (132,384)

    w_v = np.ones(NV)
    w_v[1:NV - 1] = 2.0
    RC = (Cm[:, :NV] * w_v[None, :]).T.astype(np.float32)      # (67,128)
    RS = (-Sm[:, :NV] * w_v[None, :]).T.astype(np.float32)
    rcs = np.concatenate([RC, RS], axis=1).astype(np.float32)  # (67,256)

    return {
        "kdd_packed": kdd_packed.astype(np.float32),
        "gt": gt,
        "neg67": neg67,
        "cst_hi": np.ascontiguousarray(cst[:128]),
        "cst_lo": np.ascontiguousarray(cst[128:]),
        "rcs": rcs,
    }


# --------------------------------------------------------------------------
# device kernel
# --------------------------------------------------------------------------

def build_nc(n_chan=CPC, n_batch=B, gelu=True):
    act_fn = AF.Gelu if gelu else AF.Copy
    n_img = n_chan * n_batch
    nc = bacc.Bacc("TRN2", target_bir_lowering=False, debug=False,
                   enable_asserts=False)

    x_t = nc.dram_tensor("x", [n_img, N0, N0], F32, kind="ExternalInput")
    kdd_t = nc.dram_tensor("kdd", [n_chan, NP, 3 * 4 * NV], F32,
                           kind="ExternalInput")
    gt_t = nc.dram_tensor("gt", [128, 2 * NP], F32, kind="ExternalInput")
    neg67_t = nc.dram_tensor("neg67", [128, NV], F32, kind="ExternalInput")
    csth_t = nc.dram_tensor("cst_hi", [128, 384], F32, kind="ExternalInput")
    cstl_t = nc.dram_tensor("cst_lo", [4, 384], F32, kind="ExternalInput")
    rcs_t = nc.dram_tensor("rcs", [NV, 256], F32, kind="ExternalInput")
    out_t = nc.dram_tensor("out", [n_img, 2 * N0, 2 * N0], F32,
                           kind="ExternalOutput")

    PH4 = 4 * NV          # 268
    with tile.TileContext(nc) as tc:
        with (
            tc.tile_pool(name="consts", bufs=1) as cpool,
            tc.tile_pool(name="kdd", bufs=2) as kpool,
            tc.tile_pool(name="xin", bufs=3) as xpool,
            tc.tile_pool(name="r1", bufs=2) as r1pool,
            tc.tile_pool(name="ylo", bufs=2) as ylopool,
            tc.tile_pool(name="prod", bufs=2) as prodpool,
            tc.tile_pool(name="fx", bufs=2) as fxpool,
            tc.tile_pool(name="t1", bufs=2) as t1pool,
            tc.tile_pool(name="osb", bufs=2) as opool,
            tc.tile_pool(name="ppa", bufs=2, space="PSUM") as ppa_pool,
            tc.tile_pool(name="ppy", bufs=2, space="PSUM") as ppy_pool,
            tc.tile_pool(name="pt1", bufs=1, space="PSUM") as pt1_pool,
            tc.tile_pool(name="ppd", bufs=2, space="PSUM") as ppd_pool,
        ):
            gt = cpool.tile([128, 2 * NP], F32)
            nc.sync.dma_start(gt[:], gt_t[:])
            neg67 = cpool.tile([128, NV], F32)
            nc.sync.dma_start(neg67[:], neg67_t[:])
            cst_hi = cpool.tile([128, 384], F32)
            nc.sync.dma_start(cst_hi[:], csth_t[:])
            cst_lo = cpool.tile([4, 384], F32)
            nc.sync.dma_start(cst_lo[:], cstl_t[:])
            rcs = cpool.tile([NV, 256], F32)
            nc.sync.dma_start(rcs[:], rcs_t[:])

            for ci in range(n_chan):
                k_hi = kpool.tile([128, 3 * PH4], F32, tag="k_hi")
                nc.sync.dma_start(k_hi[:], kdd_t[ci, 0:128])
                k_lo = kpool.tile([4, 3 * PH4], F32, tag="k_lo")
                nc.sync.dma_start(k_lo[:], kdd_t[ci, 128:NP])

                for bi in range(n_batch):
                    img = ci * n_batch + bi

                    x_tile = xpool.tile([N0, N0], F32, tag="x")
                    nc.sync.dma_start(x_tile[:], x_t[img])

                    # ---- stage A: R1^T = x^T @ [Gr^T | Gi^T]  (PSUM) ----
                    pA = ppa_pool.tile([128, 2 * NP], F32, tag="pA")
                    nc.tensor.matmul(pA[:], x_tile[:], gt[:],
                                     start=True, stop=True)
                    r1 = r1pool.tile([128, 2 * NP], F32, tag="r1")
                    nc.scalar.activation(r1[:], pA[:], AF.Copy)

                    # ---- stage B: Y = R1 @ G^T, cols 0..66 ----
                    # pY layout: [:,0:67]=Yr_hi  [:,67:134]=Yi_hi
                    #            [0:4,134:201]=Yr_lo  [0:4,201:268]=Yi_lo
                    pY = ppy_pool.tile([128, PH4], F32, tag="pY")
                    nc.tensor.matmul(pY[:, 0:NV], r1[:, 0:128],
                                     gt[:, 0:NV], start=True, stop=False)
                    nc.tensor.matmul(pY[:, 0:NV], r1[:, NP:NP + 128],
                                     neg67[:], start=False, stop=True)
                    nc.tensor.matmul(pY[:, NV:2 * NV], r1[:, 0:128],
                                     gt[:, NP:NP + NV], start=True, stop=False)
                    nc.tensor.matmul(pY[:, NV:2 * NV], r1[:, NP:NP + 128],
                                     gt[:, 0:NV], start=False, stop=True)
                    nc.tensor.matmul(pY[0:4, 2 * NV:3 * NV], r1[:, 128:NP],
                                     gt[:, 0:NV], start=True, stop=False)
                    nc.tensor.matmul(pY[0:4, 2 * NV:3 * NV], r1[:, NP + 128:2 * NP],
                                     neg67[:], start=False, stop=True)
                    nc.tensor.matmul(pY[0:4, 3 * NV:4 * NV], r1[:, 128:NP],
                                     gt[:, NP:NP + NV], start=True, stop=False)
                    nc.tensor.matmul(pY[0:4, 3 * NV:4 * NV], r1[:, NP + 128:2 * NP],
                                     gt[:, 0:NV], start=False, stop=True)

                    # Y lo rows to SBUF for gpsimd (gpsimd cannot read PSUM)
                    ylo = ylopool.tile([4, 2 * NV], F32, tag="ylo")
                    nc.scalar.activation(ylo[:], pY[0:4, 2 * NV:4 * NV], AF.Copy)

                    # ---- FX = Kdd_hat * Y, per phase (pointwise cmul) ----
                    # hi rows on DVE, reading Y straight from PSUM
                    y_hi_b = (pY[:, 0:2 * NV]
                              .rearrange("p (a v) -> p a v", a=2)
                              [:, :, None, :]
                              .broadcast_to([128, 2, 4, NV]))
                    pa_hi = prodpool.tile([128, 2 * PH4], F32, tag="pa_hi")
                    nc.vector.tensor_mul(
                        pa_hi[:].rearrange("p (a f v) -> p a f v", a=2, f=4),
                        k_hi[:, 0:2 * PH4].rearrange("p (a f v) -> p a f v",
                                                     a=2, f=4),
                        y_hi_b)
                    pb_hi = prodpool.tile([128, 2 * PH4], F32, tag="pb_hi")
                    nc.vector.tensor_mul(
                        pb_hi[:].rearrange("p (a f v) -> p a f v", a=2, f=4),
                        k_hi[:, PH4:3 * PH4].rearrange("p (a f v) -> p a f v",
                                                       a=2, f=4),
                        y_hi_b)
                    fxr_hi = fxpool.tile([128, PH4], F32, tag="fxr_hi")
                    nc.vector.tensor_sub(fxr_hi[:], pa_hi[:, 0:PH4],
                                         pa_hi[:, PH4:2 * PH4])
                    fxi_hi = fxpool.tile([128, PH4], F32, tag="fxi_hi")
                    nc.vector.tensor_add(fxi_hi[:], pb_hi[:, 0:PH4],
                                         pb_hi[:, PH4:2 * PH4])

                    # lo rows (u=128..131) on GPSIMD
                    y_lo_b = (ylo[:]
                              .rearrange("p (a v) -> p a v", a=2)
                              [:, :, None, :]
                              .broadcast_to([4, 2, 4, NV]))
                    pa_lo = prodpool.tile([4, 2 * PH4], F32, tag="pa_lo")
                    nc.gpsimd.tensor_mul(
                        pa_lo[:].rearrange("p (a f v) -> p a f v", a=2, f=4),
                        k_lo[:, 0:2 * PH4].rearrange("p (a f v) -> p a f v",
                                                     a=2, f=4),
                        y_lo_b)
                    pb_lo = prodpool.tile([4, 2 * PH4], F32, tag="pb_lo")
                    nc.gpsimd.tensor_mul(
                        pb_lo[:].rearrange("p (a f v) -> p a f v", a=2, f=4),
                        k_lo[:, PH4:3 * PH4].rearrange("p (a f v) -> p a f v",
                                                       a=2, f=4),
                        y_lo_b)
                    fxr_lo = fxpool.tile([4, PH4], F32, tag="fxr_lo")
                    nc.gpsimd.tensor_sub(fxr_lo[:], pa_lo[:, 0:PH4],
                                         pa_lo[:, PH4:2 * PH4])
                    fxi_lo = fxpool.tile([4, PH4], F32, tag="fxi_lo")
                    nc.gpsimd.tensor_add(fxi_lo[:], pb_lo[:, 0:PH4],
                                         pb_lo[:, PH4:2 * PH4])

                    # ---- stage C': T1^T[v,x] per phase (PSUM [67,512]) ----
                    pT1r = pt1_pool.tile([NV, 512], F32, tag="pT1r")
                    pT1i = pt1_pool.tile([NV, 512], F32, tag="pT1i")
                    for p in range(4):
                        o = pT1r[:, ts(p, 128)]
                        nc.tensor.matmul(o, fxr_hi[:, ts(p, NV)],
                                         cst_hi[:, 0:128], start=True, stop=False)
                        nc.tensor.matmul(o, fxi_hi[:, ts(p, NV)],
                                         cst_hi[:, 256:384], start=False, stop=False)
                        nc.tensor.matmul(o, fxr_lo[:, ts(p, NV)],
                                         cst_lo[:, 0:128], start=False, stop=False)
                        nc.tensor.matmul(o, fxi_lo[:, ts(p, NV)],
                                         cst_lo[:, 256:384], start=False, stop=True)
                        o = pT1i[:, ts(p, 128)]
                        nc.tensor.matmul(o, fxi_hi[:, ts(p, NV)],
                                         cst_hi[:, 0:128], start=True, stop=False)
                        nc.tensor.matmul(o, fxr_hi[:, ts(p, NV)],
                                         cst_hi[:, 128:256], start=False, stop=False)
                        nc.tensor.matmul(o, fxi_lo[:, ts(p, NV)],
                                         cst_lo[:, 0:128], start=False, stop=False)
                        nc.tensor.matmul(o, fxr_lo[:, ts(p, NV)],
                                         cst_lo[:, 128:256], start=False, stop=True)

                    t1sb = t1pool.tile([NV, 1024], F32, tag="t1sb")
                    nc.scalar.activation(t1sb[:, 0:512], pT1r[:], AF.Copy)
                    nc.scalar.activation(t1sb[:, 512:1024], pT1i[:], AF.Copy)

                    # ---- stage D: out_p = T1r@RC + T1i@RS  (PSUM [128,512]) ----
                    pD = ppd_pool.tile([128, 512], F32, tag="pD")
                    for p in range(4):
                        o = pD[:, ts(p, 128)]
                        nc.tensor.matmul(o, t1sb[:, ts(p, 128)],
                                         rcs[:, 0:128], start=True, stop=False)
                        nc.tensor.matmul(o, t1sb[:, 512 + p * 128:512 + (p + 1) * 128],
                                         rcs[:, 128:256], start=False, stop=True)

                    # ---- gelu + phase interleave + store ----
                    oute = opool.tile([128, 256], F32, tag="oute")
                    outo = opool.tile([128, 256], F32, tag="outo")
                    nc.scalar.activation(
                        oute[:].rearrange("p (v d) -> p d v", d=2),
                        pD[:, 0:256].rearrange("p (d v) -> p d v", d=2),
                        act_fn)
                    nc.scalar.activation(
                        outo[:].rearrange("p (v d) -> p d v", d=2),
                        pD[:, 256:512].rearrange("p (d v) -> p d v", d=2),
                        act_fn)
                    orows = out_t[img].rearrange("(x d) y -> d x y", d=2)
                    nc.sync.dma_start(orows[0], oute[:])
                    nc.sync.dma_start(orows[1], outo[:])

    nc.compile()
    return nc


# --------------------------------------------------------------------------
# public entry point: full inputs in, full output out
# --------------------------------------------------------------------------

def kernel(x, weight, bias):
    global LAST_EXEC_NS
    x = np.ascontiguousarray(np.asarray(x, dtype=np.float32))
    consts = _host_constants(weight, bias)

    nc = build_nc()

    in_maps = []
    for core in range(NCORES):
        c0 = core * CPC
        xs = np.ascontiguousarray(
            x[:, c0:c0 + CPC].transpose(1, 0, 2, 3)).reshape(NIMG, N0, N0)
        in_maps.append({
            "x": xs,
            "kdd": np.ascontiguousarray(consts["kdd_packed"][c0:c0 + CPC]),
            "gt": consts["gt"],
            "neg67": consts["neg67"],
            "cst_hi": consts["cst_hi"],
            "cst_lo": consts["cst_lo"],
            "rcs": consts["rcs"],
        })

    trace = os.environ.get("KERNEL_TRACE", "0") == "1"
    tmpdir = os.environ.get("KERNEL_TMPDIR") or None
    res = run_bass_kernel_spmd(nc, in_maps, list(range(NCORES)), trace=trace,
                               tmpdir=tmpdir)
    LAST_EXEC_NS = res.exec_time_ns

    out = np.empty((B, C, 2 * N0, 2 * N0), dtype=np.float32)
    for core in range(NCORES):
        c0 = core * CPC
        o = res.results[core]["out"].reshape(CPC, B, 2 * N0, 2 * N0)
        out[:, c0:c0 + CPC] = o.transpose(1, 0, 2, 3)
    return out



# revision 20
# speedup vs baseline: 2.5191x; 2.5191x over previous
"""Converse2D-Up (FFT deconvolution upsampler) as a Bass/Tile kernel for TRN2.

Math (validated against the jax reference): the whole pipeline before the
final gelu is linear in x and channel-wise.  With xp = wrap-pad(x) (132x132),
Y = FFT132(xp) = G @ x @ G^T where G = F132 @ P (132x128).  The reference's
264-point spectral transfer function H (built from weight/bias only) is
Hermitian, so out decomposes into 4 polyphase outputs
out_dd = real(IFFT132(Kdd_hat . Y)) with per-channel precomputed spectra
Kdd_hat; the crop leaves exactly 128 rows/cols per phase.  Hermitian symmetry
means only columns v=0..66 of Kdd_hat.Y are needed:
    T1[v,x] = sum_u (Kdd_hat.Y)[u,pv] Ai[x,u]
    out[x,y] = sum_{v} w_v (T1r[v,x] RC[v,y] + T1i[v,x] RS[v,y])

v2: all PE matmuls in bf16 (1 cycle/row vs 4 for fp32), packed N>=256 moving
operands (29 matmuls/image vs 98), the 4 low rows (u=128..131) of the
pointwise complex multiply batched across the 4 images of a channel, and the
gelu+interleave output written with a single contiguous DMA per image.

Sharding: 8 channels per core x 4 batch images; weight/bias-derived spectra
are host-precomputed constants.
"""

import os

import ml_dtypes
import numpy as np

import concourse.bass as bass
import concourse.mybir as mybir
import concourse.tile as tile
from concourse import bacc
from concourse.bass import ts
from concourse.bass_utils import run_bass_kernel_spmd

F32 = mybir.dt.float32
BF16 = mybir.dt.bfloat16
AF = mybir.ActivationFunctionType

SCALE = 2
PAD = 2
EPS = 1e-5
N0 = 128           # input spatial size
NP = N0 + 2 * PAD  # 132 padded
NU = NP * SCALE    # 264 upsampled
NV = NP // 2 + 1   # 67 unique spectral columns
B = 4
C = 64
NCORES = 8
CPC = C // NCORES  # 8 channels per core
NIMG = B * CPC     # 32 images per core

LAST_EXEC_NS = None  # set by kernel() when tracing is enabled


# --------------------------------------------------------------------------
# host-side constant precompute (weight/bias -> per-channel spectra)
# --------------------------------------------------------------------------

def _host_constants(weight, bias):
    w64 = np.asarray(weight, dtype=np.float64)
    b64 = np.asarray(bias, dtype=np.float64)

    # FB = p2o(weight): 264-point OTF of the rolled 3x3 PSF, per channel
    k_h, k_w = w64.shape[-2:]
    otf = np.zeros((C, NU, NU), dtype=np.complex128)
    otf[:, :k_h, :k_w] = w64[0]
    otf = np.roll(otf, (-(k_h // 2), -(k_w // 2)), axis=(-2, -1))
    FB = np.fft.fftn(otf, axes=(-2, -1))                      # (C,264,264)

    biaseps = 1.0 / (1.0 + np.exp(-(b64.reshape(C) - 9.0))) + EPS  # (C,)
    be = biaseps[:, None, None]

    u = np.arange(NU)
    Dr = 1 + np.exp(-2j * np.pi * u / NU)
    D = Dr[:, None] * Dr[None, :]                             # (264,264)

    Gh = np.conj(FB) + be * D[None]
    FBG = FB * Gh

    def quadmean(A):
        return 0.25 * (A[:, :NP, :NP] + A[:, NP:, :NP]
                       + A[:, :NP, NP:] + A[:, NP:, NP:])

    M1 = quadmean(FBG)
    invW = quadmean(np.abs(FB) ** 2)
    M2 = M1 / (invW + be)
    H = (Gh - np.conj(FB) * np.tile(M2, (1, SCALE, SCALE))) / be   # (C,264,264)

    hr = np.fft.ifft2(H, axes=(-2, -1)).real                  # H Hermitian
    # polyphase spectra: Kdd_hat[c,dx,dy] = FFT132(hr[c, dx::2, dy::2])
    kdd = np.empty((C, 2, 2, NP, NV), dtype=np.complex128)
    for dx in range(2):
        for dy in range(2):
            kh = np.fft.fft2(hr[:, dx::2, dy::2], axes=(-2, -1))
            kdd[:, dx, dy] = kh[:, :, :NV]

    # forward matrix G = F132 @ P  (132x128 complex)
    P = np.zeros((NP, N0))
    for m in range(NP):
        P[m, (m - PAD) % N0] = 1.0
    F132 = np.exp(-2j * np.pi * np.outer(np.arange(NP), np.arange(NP)) / NP)
    G = F132 @ P

    # inverse matrix, rows i in [2,130) of iF132/132
    Ai = np.exp(2j * np.pi * np.outer(np.arange(2, 130), np.arange(NP)) / NP) / NP
    Cm, Sm = Ai.real, Ai.imag                                  # (128,132)
    CT, ST = Cm.T, Sm.T                                        # (132,128)

    w_v = np.ones(NV)
    w_v[1:NV - 1] = 2.0
    RC = (Cm[:, :NV] * w_v[None, :]).T                         # (67,128)
    RS = (-Sm[:, :NV] * w_v[None, :]).T

    bf = ml_dtypes.bfloat16

    # per-channel spectra packed for the DVE complex multiply:
    # cols (a, p, v): a in {0,1}; set1=(Kr,Ki), set2=(Ki,Kr)
    kr = np.ascontiguousarray(
        kdd.real.transpose(0, 3, 1, 2, 4)).reshape(C, NP, 4 * NV)
    ki = np.ascontiguousarray(
        kdd.imag.transpose(0, 3, 1, 2, 4)).reshape(C, NP, 4 * NV)
    khi = np.concatenate([kr[:, :128], ki[:, :128],
                          ki[:, :128], kr[:, :128]], axis=2)   # (C,128,1072)
    # lo rows (u=128..131) live at partition strips {0,32,64,96}, one strip
    # per batch image, so the stationary slices satisfy the PE tile_position
    # alignment; spectra replicated across strips
    klo_s = np.concatenate([kr[:, 128:], ki[:, 128:],
                            ki[:, 128:], kr[:, 128:]], axis=2)   # (C,4,1072)
    klo = np.zeros((C, 100, 1072), klo_s.dtype)
    for b in range(4):
        klo[:, 32 * b:32 * b + 4] = klo_s

    return {
        "gt264": np.concatenate([G.real.T, G.imag.T], 1).astype(bf),
        "bgt1": np.concatenate([G.real[0:NV].T, G.imag[0:NV].T], 1).astype(bf),
        "bgt2": np.concatenate([-G.imag[0:NV].T, G.real[0:NV].T], 1).astype(bf),
        "cst1": np.concatenate([CT[:128], ST[:128]], 1).astype(bf),
        "cst2": np.concatenate([-ST[:128], CT[:128]], 1).astype(bf),
        "cst1lo": np.tile(np.concatenate([CT[128:], ST[128:]], 1),
                          (25, 1))[:100].astype(bf),
        "cst2lo": np.tile(np.concatenate([-ST[128:], CT[128:]], 1),
                          (25, 1))[:100].astype(bf),
        "rcs": np.concatenate([RC, RS], 1).astype(bf),
        "rcs2": np.concatenate([RS, RC], 1).astype(bf),
        "khi": khi.astype(bf),
        "klo": klo.astype(bf),
    }


# --------------------------------------------------------------------------
# device kernel
# --------------------------------------------------------------------------

def build_nc():
    nc = bacc.Bacc("TRN2", target_bir_lowering=False, debug=False,
                   enable_asserts=False)

    x_t = nc.dram_tensor("x", [NIMG, N0, N0], BF16, kind="ExternalInput")
    khi_t = nc.dram_tensor("khi", [CPC, 128, 1072], BF16, kind="ExternalInput")
    klo_t = nc.dram_tensor("klo", [CPC, 100, 1072], BF16, kind="ExternalInput")
    gt264_t = nc.dram_tensor("gt264", [128, 2 * NP], BF16, kind="ExternalInput")
    bgt1_t = nc.dram_tensor("bgt1", [128, 2 * NV], BF16, kind="ExternalInput")
    bgt2_t = nc.dram_tensor("bgt2", [128, 2 * NV], BF16, kind="ExternalInput")
    cst1_t = nc.dram_tensor("cst1", [128, 256], BF16, kind="ExternalInput")
    cst2_t = nc.dram_tensor("cst2", [128, 256], BF16, kind="ExternalInput")
    cst1lo_t = nc.dram_tensor("cst1lo", [100, 256], BF16, kind="ExternalInput")
    cst2lo_t = nc.dram_tensor("cst2lo", [100, 256], BF16, kind="ExternalInput")
    rcs_t = nc.dram_tensor("rcs", [NV, 256], BF16, kind="ExternalInput")
    rcs2_t = nc.dram_tensor("rcs2", [NV, 256], BF16, kind="ExternalInput")
    out_t = nc.dram_tensor("out", [NIMG, 2 * N0, 2 * N0], F32,
                           kind="ExternalOutput")

    PH4 = 4 * NV          # 268
    with tile.TileContext(nc) as tc:
        with (
            tc.tile_pool(name="consts", bufs=1) as cpool,
            tc.tile_pool(name="kdd", bufs=2) as kpool,
            tc.tile_pool(name="xin", bufs=4) as xpool,
            tc.tile_pool(name="r1", bufs=2) as r1pool,
            tc.tile_pool(name="yev", bufs=2) as ypool,
            tc.tile_pool(name="prod", bufs=2) as papool,
            tc.tile_pool(name="fx", bufs=2) as fxpool,
            tc.tile_pool(name="t1", bufs=2) as t1pool,
            tc.tile_pool(name="osb", bufs=2) as opool,
            tc.tile_pool(name="ppa", bufs=1, space="PSUM") as ppa_pool,
            tc.tile_pool(name="ppy", bufs=1, space="PSUM") as ppy_pool,
            tc.tile_pool(name="pt1", bufs=2, space="PSUM") as pt1_pool,
            tc.tile_pool(name="ppd", bufs=1, space="PSUM") as ppd_pool,
        ):
            gt264 = cpool.tile([128, 2 * NP], BF16)
            nc.sync.dma_start(gt264[:], gt264_t[:])
            bgt1 = cpool.tile([128, 2 * NV], BF16)
            nc.sync.dma_start(bgt1[:], bgt1_t[:])
            bgt2 = cpool.tile([128, 2 * NV], BF16)
            nc.sync.dma_start(bgt2[:], bgt2_t[:])
            cst1 = cpool.tile([128, 256], BF16)
            nc.sync.dma_start(cst1[:], cst1_t[:])
            cst2 = cpool.tile([128, 256], BF16)
            nc.sync.dma_start(cst2[:], cst2_t[:])
            cst1lo = cpool.tile([100, 256], BF16)
            nc.sync.dma_start(cst1lo[:], cst1lo_t[:])
            cst2lo = cpool.tile([100, 256], BF16)
            nc.sync.dma_start(cst2lo[:], cst2lo_t[:])
            rcs = cpool.tile([NV, 256], BF16)
            nc.sync.dma_start(rcs[:], rcs_t[:])
            rcs2 = cpool.tile([NV, 256], BF16)
            nc.sync.dma_start(rcs2[:], rcs2_t[:])

            for ci in range(CPC):
                khi = kpool.tile([128, 1072], BF16, tag="khi")
                nc.sync.dma_start(khi[:], khi_t[ci])
                klo = kpool.tile([100, 1072], BF16, tag="klo")
                nc.sync.dma_start(klo[:], klo_t[ci])
                ylop = ypool.tile([100, 2 * NV], BF16, tag="ylop")

                fx_list = []
                for bi in range(B):
                    img = ci * B + bi

                    x_tile = xpool.tile([N0, N0], BF16, tag="x")
                    nc.gpsimd.dma_start(x_tile[:], x_t[img])

                    # ---- stage A: R1^T = x^T @ [Gr^T | Gi^T]  (PSUM) ----
                    pA = ppa_pool.tile([128, 2 * NP], F32, tag="pA")
                    nc.tensor.matmul(pA[:], x_tile[:], gt264[:],
                                     start=True, stop=True)
                    r1 = r1pool.tile([128, 2 * NP], BF16, tag="r1")
                    nc.scalar.copy(r1[:], pA[:])

                    # ---- stage B: Y cols 0..66 (hi rows + 4 lo rows) ----
                    pY = ppy_pool.tile([128, 2 * PH4 // 2], F32, tag="pY")
                    nc.tensor.matmul(pY[:, 0:2 * NV], r1[:, 0:128],
                                     bgt1[:], start=True, stop=False)
                    nc.tensor.matmul(pY[:, 0:2 * NV], r1[:, NP:NP + 128],
                                     bgt2[:], start=False, stop=True)
                    nc.tensor.matmul(pY[0:4, 2 * NV:4 * NV], r1[:, 128:NP],
                                     bgt1[:], start=True, stop=False)
                    nc.tensor.matmul(pY[0:4, 2 * NV:4 * NV],
                                     r1[:, NP + 128:2 * NP],
                                     bgt2[:], start=False, stop=True)

                    yall = ypool.tile([128, 2 * NV], BF16, tag="yall")
                    nc.scalar.copy(yall[:], pY[:, 0:2 * NV])
                    nc.scalar.copy(ylop[32 * bi:32 * bi + 4, :],
                                   pY[0:4, 2 * NV:4 * NV])

                    # ---- FX = Kdd_hat * Y (hi rows), DVE + gpsimd ----
                    y_b = (yall[:]
                           .rearrange("p (a v) -> p a v", a=2)
                           [:, :, None, :]
                           .broadcast_to([128, 2, 4, NV]))
                    pa = papool.tile([128, 2 * PH4], BF16, tag="pa")
                    nc.vector.tensor_mul(
                        pa[:].rearrange("p (a f v) -> p a f v", a=2, f=4),
                        khi[:, 0:2 * PH4].rearrange("p (a f v) -> p a f v",
                                                    a=2, f=4),
                        y_b)
                    pb = papool.tile([128, 2 * PH4], BF16, tag="pb")
                    nc.vector.tensor_mul(
                        pb[:].rearrange("p (a f v) -> p a f v", a=2, f=4),
                        khi[:, 2 * PH4:4 * PH4].rearrange(
                            "p (a f v) -> p a f v", a=2, f=4),
                        y_b)
                    fxr = fxpool.tile([128, PH4], BF16, tag=f"fxr{bi}")
                    nc.gpsimd.tensor_sub(fxr[:], pa[:, 0:PH4],
                                         pa[:, PH4:2 * PH4])
                    fxi = fxpool.tile([128, PH4], BF16, tag=f"fxi{bi}")
                    nc.vector.tensor_add(fxi[:], pb[:, 0:PH4],
                                         pb[:, PH4:2 * PH4])
                    fx_list.append((fxr, fxi))

                # ---- lo rows (u=128..131), images at partition strips ----
                ylo_b = (ylop[:]
                         .rearrange("p (a v) -> p a v", a=2)
                         [:, :, None, :]
                         .broadcast_to([100, 2, 4, NV]))
                palo = papool.tile([100, 2 * PH4], BF16, tag="palo")
                nc.vector.tensor_mul(
                    palo[:].rearrange("p (a f v) -> p a f v", a=2, f=4),
                    klo[:, 0:2 * PH4].rearrange("p (a f v) -> p a f v",
                                                a=2, f=4),
                    ylo_b)
                pblo = papool.tile([100, 2 * PH4], BF16, tag="pblo")
                nc.vector.tensor_mul(
                    pblo[:].rearrange("p (a f v) -> p a f v", a=2, f=4),
                    klo[:, 2 * PH4:4 * PH4].rearrange("p (a f v) -> p a f v",
                                                      a=2, f=4),
                    ylo_b)
                fxlr = fxpool.tile([100, PH4], BF16, tag="fxlr")
                nc.gpsimd.tensor_sub(fxlr[:], palo[:, 0:PH4],
                                     palo[:, PH4:2 * PH4])
                fxli = fxpool.tile([100, PH4], BF16, tag="fxli")
                nc.vector.tensor_add(fxli[:], pblo[:, 0:PH4],
                                     pblo[:, PH4:2 * PH4])

                for bi in range(B):
                    img = ci * B + bi
                    fxr, fxi = fx_list[bi]

                    # ---- stage C': T1 per phase, packed rhs N=256 ----
                    # pt1 cols p*256+[0:128]=T1r_p, +[128:256]=T1i_p
                    pt1 = pt1_pool.tile([NV, 1024], F32, tag="pt1")
                    for p in range(4):
                        o = pt1[:, ts(p, 256)]
                        nc.tensor.matmul(o, fxr[:, ts(p, NV)], cst1[:],
                                         start=True, stop=False)
                        nc.tensor.matmul(o, fxi[:, ts(p, NV)], cst2[:],
                                         start=False, stop=False)
                        s0 = 32 * bi
                        nc.tensor.matmul(o, fxlr[s0:s0 + 4, ts(p, NV)],
                                         cst1lo[s0:s0 + 4, :],
                                         start=False, stop=False,
                                         tile_position=(s0, 0))
                        nc.tensor.matmul(o, fxli[s0:s0 + 4, ts(p, NV)],
                                         cst2lo[s0:s0 + 4, :],
                                         start=False, stop=True,
                                         tile_position=(s0, 0))

                    t1 = t1pool.tile([NV, 1024], BF16, tag="t1")
                    nc.scalar.copy(t1[:, 0:448], pt1[:, 0:448])
                    nc.vector.tensor_copy(t1[:, 448:1024], pt1[:, 448:1024])

                    # ---- stage D: out_p = T1r_p@RC + T1i_p@RS ----
                    # rhs [RC|RS] / [RS|RC] keeps N=256 so LDWEIGHTS hides;
                    # the second half of each 256-block is discarded
                    pD = ppd_pool.tile([128, 1024], F32, tag="pD")
                    for p in range(4):
                        o = pD[:, ts(p, 256)]
                        nc.tensor.matmul(o, t1[:, p * 256:p * 256 + 128],
                                         rcs[:], start=True, stop=False)
                        nc.tensor.matmul(o, t1[:, p * 256 + 128:(p + 1) * 256],
                                         rcs2[:], start=False, stop=True)

                    # ---- gelu + phase interleave + single DMA ----
                    pDv = pD[:].rearrange("q (p s v) -> q p s v", p=4, s=2)
                    osb = opool.tile([128, 512], F32, tag="osb")
                    nc.scalar.activation(
                        osb[:, 0:256].rearrange("p (v d) -> p d v", d=2),
                        pDv[:, 0:2, 0, :],
                        AF.Gelu)
                    nc.scalar.activation(
                        osb[:, 256:512].rearrange("p (v d) -> p d v", d=2),
                        pDv[:, 2:4, 0, :],
                        AF.Gelu)
                    nc.sync.dma_start(
                        out_t[img].rearrange("(x d) y -> x (d y)", d=2),
                        osb[:])

    nc.compile()
    return nc


# --------------------------------------------------------------------------
# public entry point: full inputs in, full output out
# --------------------------------------------------------------------------

def kernel(x, weight, bias):
    global LAST_EXEC_NS
    x = np.asarray(x, dtype=np.float32)
    consts = _host_constants(weight, bias)

    nc = build_nc()

    bf = ml_dtypes.bfloat16
    in_maps = []
    for core in range(NCORES):
        c0 = core * CPC
        xs = np.ascontiguousarray(
            x[:, c0:c0 + CPC].transpose(1, 0, 2, 3)).reshape(
                NIMG, N0, N0).astype(bf)
        in_maps.append({
            "x": xs,
            "khi": np.ascontiguousarray(consts["khi"][c0:c0 + CPC]),
            "klo": np.ascontiguousarray(consts["klo"][c0:c0 + CPC]),
            "gt264": consts["gt264"],
            "bgt1": consts["bgt1"],
            "bgt2": consts["bgt2"],
            "cst1": consts["cst1"],
            "cst2": consts["cst2"],
            "cst1lo": consts["cst1lo"],
            "cst2lo": consts["cst2lo"],
            "rcs": consts["rcs"],
            "rcs2": consts["rcs2"],
        })

    trace = os.environ.get("KERNEL_TRACE", "0") == "1"
    tmpdir = os.environ.get("KERNEL_TMPDIR") or None
    res = run_bass_kernel_spmd(nc, in_maps, list(range(NCORES)), trace=trace,
                               tmpdir=tmpdir)
    LAST_EXEC_NS = res.exec_time_ns

    out = np.empty((B, C, 2 * N0, 2 * N0), dtype=np.float32)
    for core in range(NCORES):
        c0 = core * CPC
        o = res.results[core]["out"].reshape(CPC, B, 2 * N0, 2 * N0)
        out[:, c0:c0 + CPC] = o.transpose(1, 0, 2, 3)
    return out
